# revision 1
# baseline (speedup 1.0000x reference)
"""Trainium2 Bass kernel for nn_Attention2 (sparse additive attention).

Math (per batch b):
    att_h  = h @ W_h2att.T + b_h2att                       [HID]
    dot    = tanh(p_att_feats[b] + att_h)                  [S, HID]
    scores = dot @ w_alpha (+ b_alpha, cancels in softmax) [S]
    scores = where(mask, -1e8, scores)
    w      = softmax(scores)          (masked rows get weight exactly 0)
    out[b] = w @ att_feats[b]                              [RNN]

Strategy: pure data parallel over batch (16 batches / core on 8 cores).
Rows (b, s) with mask==True contribute exactly zero (exp(-1e8) == 0 in
fp32), so the kernel gathers only unmasked rows (one indirect DMA per
128-row chunk over host-fused [p | A] rows), packs them densely into
chunks that may span batches, and uses host-built one-hot batch
matrices for segmented reductions on the tensor engine:
    bc   = oht.T @ att_h       (per-row broadcast of its batch's att_h)
    x    = bc + p                                   [DVE]
    t    = tanh(x)                                  [ACT]
    s    = reduce(t * w_alpha_bcast)                [DVE mul + reduce]
    e    = exp(s)                                   [ACT]
    ohw  = oh * e                                   [DVE]
    res  += ohw.T @ A_rows ; sums += ohw.T @ ones   [PE, psum accum]
    out  = res / sums
b_alpha and the softmax max-subtraction cancel and are omitted.

dt_mode: "f32" (exact), "f32r" (PE streams fp32 in 1 pass, ~1e-4 err),
"bf16" (gathered data + matmul operands in bf16, ~5e-4 err, halves DMA).
"""

import os
import sys
from contextlib import ExitStack

import numpy as np

for _p in (
    "/root/.axon_site",
    "/root/.axon_site/_ro/trn_rl_repo",
    "/root/.axon_site/_ro/pypackages",
    "/opt/trn_rl_repo",
):
    if os.path.isdir(_p) and _p not in sys.path:
        sys.path.append(_p)

import ml_dtypes
import concourse.bass as bass
import concourse.tile as tile
from concourse import bacc, mybir
from concourse.bass import IndirectOffsetOnAxis
from concourse.bass_utils import run_bass_kernel_spmd

B, S, RNN, HID = 128, 1024, 1024, 512
NCORES = 8
BS = B // NCORES  # batches per core
P = 128
F32 = mybir.dt.float32
F32R = mybir.dt.float32r
BF16 = mybir.dt.bfloat16
I32 = mybir.dt.int32
KCHUNKS = RNN // P  # k-chunks for the att_h matmul

_DT = {"f32": F32, "f32r": F32R, "bf16": BF16}
_NPDT = {"f32": np.float32, "f32r": np.float32, "bf16": ml_dtypes.bfloat16}


_PROG_CACHE = {}


def _build_program(nchunks: int, use_gather: bool, dt_mode: str):
    key = (nchunks, use_gather, dt_mode)
    if key in _PROG_CACHE:
        return _PROG_CACHE[key]
    nc = bacc.Bacc("TRN2", target_bir_lowering=False, debug=False, num_devices=NCORES)
    mmdt = _DT[dt_mode]
    bf = dt_mode == "bf16"

    pA_d = nc.dram_tensor("pA", [BS * S, HID + RNN], mmdt, kind="ExternalInput").ap()
    hT_d = nc.dram_tensor("hT", [RNN, BS], F32, kind="ExternalInput").ap()
    wT_d = nc.dram_tensor("wT", [RNN, HID], F32, kind="ExternalInput").ap()
    bias_d = nc.dram_tensor("bias", [1, HID], F32, kind="ExternalInput").ap()
    wab_d = nc.dram_tensor("wab", [P, HID], mmdt, kind="ExternalInput").ap()
    oh_d = nc.dram_tensor("oh", [nchunks, P, BS], mmdt, kind="ExternalInput").ap()
    oht_d = nc.dram_tensor("oht", [nchunks, BS, P], mmdt, kind="ExternalInput").ap()
    ones_d = nc.dram_tensor("ones", [P, 8], mmdt, kind="ExternalInput").ap()
    if use_gather:
        idx_d = nc.dram_tensor("idx", [nchunks, P], I32, kind="ExternalInput").ap()
    out_d = nc.dram_tensor("out", [BS, RNN], F32, kind="ExternalOutput").ap()

    with tile.TileContext(nc) as tc, ExitStack() as ctx:
        const = ctx.enter_context(tc.tile_pool(name="const", bufs=1))
        loads = ctx.enter_context(tc.tile_pool(name="loads", bufs=8 if bf else 4))
        work = ctx.enter_context(tc.tile_pool(name="work", bufs=8 if bf else 3))
        small = ctx.enter_context(tc.tile_pool(name="small", bufs=12))
        ps_work = ctx.enter_context(tc.tile_pool(name="ps_work", bufs=3, space="PSUM"))
        ps_hold = ctx.enter_context(tc.tile_pool(name="ps_hold", bufs=1, space="PSUM"))

        # ---- setup: constants + att_h = h @ W.T + bias ----
        wT_sb = const.tile([P, KCHUNKS, HID], F32)
        nc.sync.dma_start(out=wT_sb, in_=wT_d.rearrange("(j p) d -> p j d", p=P))
        hT_sb = const.tile([P, KCHUNKS, BS], F32)
        nc.sync.dma_start(out=hT_sb, in_=hT_d.rearrange("(j p) b -> p j b", p=P))
        bias_sb = const.tile([1, HID], F32)
        nc.sync.dma_start(out=bias_sb, in_=bias_d)
        wab_sb = const.tile([P, HID], mmdt)
        nc.sync.dma_start(out=wab_sb, in_=wab_d)
        ones_sb = const.tile([P, 8], mmdt)
        nc.sync.dma_start(out=ones_sb, in_=ones_d)
        onesb_sb = const.tile([1, BS], F32)
        nc.vector.memset(onesb_sb, 1.0)

        # bulk-load all per-chunk metadata once
        oh_all = const.tile([P, nchunks, BS], mmdt)
        nc.scalar.dma_start(out=oh_all, in_=oh_d.rearrange("c p b -> p c b"))
        oht_all = const.tile([BS, nchunks, P], mmdt)
        nc.scalar.dma_start(out=oht_all, in_=oht_d.rearrange("c b p -> b c p"))
        if use_gather:
            # idx rides gpsimd's own (empty) SWDGE queue, not behind the
            # 3MB of constants on Sync — gathers queue FIFO right after it.
            idx_all = const.tile([P, nchunks], I32)
            nc.gpsimd.dma_start(out=idx_all, in_=idx_d.rearrange("c p -> p c"))

        att_ps_full = ps_work.tile([P, 2, HID], F32, tag="w_ps")
        att_ps = att_ps_full[0:BS, 0, :]
        for j in range(KCHUNKS):
            nc.tensor.matmul(
                out=att_ps,
                lhsT=hT_sb[:, j, :],
                rhs=wT_sb[:, j, :],
                start=(j == 0),
                stop=False,
            )
        nc.tensor.matmul(out=att_ps, lhsT=onesb_sb, rhs=bias_sb, start=False, stop=True)
        att_h_sb = const.tile([BS, HID], mmdt)
        nc.scalar.copy(att_h_sb, att_ps)

        hold_ps = ps_hold.tile([P, RNN], F32)
        res_ps = hold_ps[0:BS, :]
        sums_ps = hold_ps[64 : 64 + BS, 0:8]

        # ---- main loop over PAIRS of packed 128-row chunks ----
        # Two chunks share one SBUF tile and one pass of each DVE/ACT op
        # (3D access patterns), halving per-op fixed overheads. The res-phase
        # matmuls are emitted LAG pairs behind the scores-phase: they depend
        # on exp (a long cross-engine chain) and the PE queue is FIFO, so
        # lagging keeps the PE fed with already-ready bc matmuls.
        LAG = 4
        pend = []  # (ohw, pA2, cs) awaiting res-phase emission

        def emit_res(ohw, pA2, cs):
            for j, c in enumerate(cs):
                A_t = pA2[:, j, HID : HID + RNN]
                st, sp = (c == 0), (c == nchunks - 1)
                nc.tensor.matmul(
                    out=res_ps[:, 0:512],
                    lhsT=ohw[:, j, :],
                    rhs=A_t[:, 0:512],
                    start=st,
                    stop=sp,
                )
                nc.tensor.matmul(
                    out=res_ps[:, 512:1024],
                    lhsT=ohw[:, j, :],
                    rhs=A_t[:, 512:1024],
                    start=st,
                    stop=sp,
                )
                nc.tensor.matmul(
                    out=sums_ps, lhsT=ohw[:, j, :], rhs=ones_sb, start=st, stop=sp
                )

        npairs = (nchunks + 1) // 2
        for pi in range(npairs):
            cs = [c for c in (2 * pi, 2 * pi + 1) if c < nchunks]
            nj = len(cs)

            pA2 = loads.tile([P, 2, HID + RNN], mmdt, tag="pA2")
            if True:
                for j, c in enumerate(cs):
                    if use_gather:
                        nc.gpsimd.indirect_dma_start(
                            out=pA2[:, j, :],
                            out_offset=None,
                            in_=pA_d,
                            in_offset=IndirectOffsetOnAxis(
                                ap=idx_all[:, c : c + 1], axis=0
                            ),
                        )
                    else:
                        nc.sync.dma_start(
                            out=pA2[:, j, :], in_=pA_d[c * P : (c + 1) * P, :]
                        )

            # bc = oht.T @ att_h per chunk (each half is one PSUM bank)
            w_ps = ps_work.tile([P, 2, HID], F32)
            for j, c in enumerate(cs):
                nc.tensor.matmul(
                    out=w_ps[:, j, :],
                    lhsT=oht_all[:, c, :],
                    rhs=att_h_sb,
                    start=True,
                    stop=True,
                )

            # x = bc + p ; tanh ; * w_alpha ; reduce — one pass per pair
            x_t = work.tile([P, 2, HID], F32, tag="x")
            nc.vector.tensor_add(
                x_t[:, :nj, :], w_ps[:, :nj, :], pA2[:, :nj, 0:HID]
            )
            tanh_t = work.tile([P, 2, HID], mmdt if bf else F32, tag="tanh")
            nc.scalar.activation(
                out=tanh_t[:, :nj, :],
                in_=x_t[:, :nj, :],
                func=mybir.ActivationFunctionType.Tanh,
            )
            scr = work.tile([P, 2, HID], mmdt if bf else F32, tag="scr")
            nc.vector.tensor_mul(
                scr[:, :nj, :], tanh_t[:, :nj, :], wab_sb.unsqueeze(1).broadcast_to([P, nj, HID])
            )
            sc_pair = small.tile([P, 2], F32, tag="sc")
            if pi % 3 == 0 and nj == 2:
                dump = work.tile([P, 2, HID], mmdt if bf else F32, tag="dump")
                for j in range(nj):
                    nc.scalar.activation(
                        out=dump[:, j, :],
                        in_=scr[:, j, :],
                        func=mybir.ActivationFunctionType.Copy,
                        accum_out=sc_pair[:, j : j + 1],
                    )
            else:
                nc.vector.tensor_reduce(
                    out=sc_pair[:, :nj],
                    in_=scr[:, :nj, :],
                    axis=mybir.AxisListType.X,
                    op=mybir.AluOpType.add,
                )

            exp_pair = small.tile([P, 2], F32, tag="exp")
            nc.scalar.activation(
                out=exp_pair[:, :nj],
                in_=sc_pair[:, :nj],
                func=mybir.ActivationFunctionType.Exp,
            )

            # ohw = oh * exp (per-partition scale) — cheap, runs on ACT
            ohw = small.tile([P, 2, BS], mmdt, tag="ohw")
            for j, c in enumerate(cs):
                nc.scalar.activation(
                    out=ohw[:, j, :],
                    in_=oh_all[:, c, :],
                    func=mybir.ActivationFunctionType.Copy,
                    scale=exp_pair[:, j : j + 1],
                )

            pend.append((ohw, pA2, cs))
            if len(pend) > LAG:
                emit_res(*pend.pop(0))
        for args in pend:
            emit_res(*args)

        # ---- normalize + store ----
        recip_sb = const.tile([BS, 1], F32)
        nc.vector.reciprocal(recip_sb, sums_ps[:, 0:1])
        out_sb = const.tile([BS, RNN], F32)
        nc.vector.tensor_scalar_mul(out=out_sb, in0=res_ps, scalar1=recip_sb)
        nc.sync.dma_start(out=out_d, in_=out_sb)

    nc.compile()
    _PROG_CACHE[key] = nc
    return nc


def _prep_core(m, h, pA_full, mask, use_gather, npdt):
    """Host-side shard prep for core m."""
    sl = slice(m * BS, (m + 1) * BS)
    mask_m = mask[sl]
    if use_gather:
        rows = np.concatenate(
            [b * S + np.flatnonzero(~mask_m[b]) for b in range(BS)]
        ).astype(np.int64)
    else:
        rows = np.arange(BS * S, dtype=np.int64)
    nch = (len(rows) + P - 1) // P
    in_map = {
        "pA": pA_full[m],
        "hT": np.ascontiguousarray(h[sl].T),
    }
    return in_map, rows, nch


def _finish_core(in_map, rows, nchunks, mask_flat_m, use_gather, npdt):
    r = len(rows)
    total = nchunks * P
    idx = np.zeros(total, np.int32)
    idx[:r] = rows
    oh = np.zeros((total, BS), np.float32)
    if use_gather:
        oh[np.arange(r), (rows // S).astype(np.int64)] = 1.0
    else:
        keep = ~mask_flat_m
        kk = np.flatnonzero(keep)
        oh[kk, (kk // S).astype(np.int64)] = 1.0
    oh = oh.reshape(nchunks, P, BS)
    oht = np.ascontiguousarray(oh.transpose(0, 2, 1))
    in_map["oh"] = oh.astype(npdt)
    in_map["oht"] = oht.astype(npdt)
    if use_gather:
        in_map["idx"] = idx.reshape(nchunks, P)
    return in_map


def run(
    inputs,
    use_gather: bool = True,
    dt_mode: str = "bf16",
    trace: bool = False,
    trace_kwargs: dict | None = None,
):
    h = np.asarray(inputs["h"], dtype=np.float32)
    A = np.asarray(inputs["att_feats"], dtype=np.float32)
    p = np.asarray(inputs["p_att_feats"], dtype=np.float32)
    mask = np.asarray(inputs["mask"]).astype(bool)
    W = np.asarray(inputs["W_h2att"], dtype=np.float32)
    bh = np.asarray(inputs["b_h2att"], dtype=np.float32)
    wa = np.asarray(inputs["w_alpha"], dtype=np.float32)
    npdt = _NPDT[dt_mode]

    # fused [p | A] rows per core, in the matmul dtype
    pA_full = []
    for m in range(NCORES):
        sl = slice(m * BS, (m + 1) * BS)
        pA = np.empty((BS * S, HID + RNN), npdt)
        pA[:, :HID] = p[sl].reshape(BS * S, HID).astype(npdt)
        pA[:, HID:] = A[sl].reshape(BS * S, RNN).astype(npdt)
        pA_full.append(pA)

    shared = {
        "wT": np.ascontiguousarray(W.T),
        "bias": np.ascontiguousarray(bh[None, :]),
        "wab": np.ascontiguousarray(
            np.broadcast_to(wa[None, :], (P, HID)).astype(npdt)
        ),
        "ones": np.ones((P, 8), npdt),
    }

    per_core = [
        _prep_core(m, h, pA_full, mask, use_gather, npdt) for m in range(NCORES)
    ]
    nchunks = max(nch for (_, _, nch) in per_core)
    in_maps = []
    for m, (in_map, rows, _) in enumerate(per_core):
        mask_flat = mask[m * BS : (m + 1) * BS].reshape(-1)
        in_map = _finish_core(in_map, rows, nchunks, mask_flat, use_gather, npdt)
        in_map.update(shared)
        in_maps.append(in_map)

    nc = _build_program(nchunks, use_gather, dt_mode)
    br = run_bass_kernel_spmd(
        nc,
        in_maps,
        core_ids=list(range(NCORES)),
        trace=trace,
        **(trace_kwargs or {}),
    )
    out = np.concatenate([br.results[m]["out"] for m in range(NCORES)], axis=0)
    out = out.astype(np.float32)

    # Fully-masked batch (measure-zero for random masks): reference softmax
    # degenerates to uniform weights over all S; the gather path would give
    # 0/0. Patch those rows host-side with the uniform average.
    dead = np.flatnonzero(mask.all(axis=1))
    for b in dead:
        out[b] = A[b].mean(axis=0, dtype=np.float64).astype(np.float32)
    return out, br


def kernel(**inputs) -> np.ndarray:
    use_gather = os.environ.get("ATT_USE_GATHER", "1") == "1"
    dt_mode = os.environ.get("ATT_DT_MODE", "bf16")
    out, _ = run(inputs, use_gather=use_gather, dt_mode=dt_mode, trace=False)
    return out



# revision 7
# speedup vs baseline: 1.5917x; 1.5917x over previous
"""Trainium2 Bass kernel for nn_Attention2 (sparse additive attention), v2.

Math (per batch b):
    att_h  = h @ W_h2att.T + b_h2att                       [HID]
    x      = p_att_feats[b] + att_h                        [S, HID]
    scores = tanh(x) @ w_alpha   (+ b_alpha, cancels)      [S]
    scores = where(mask, -1e8, scores)
    w      = softmax(scores)
    out[b] = w @ att_feats[b]                              [RNN]

Data-parallel over batch (16 batches/core on 8 cores). Only unmasked rows
are processed; the host packs them densely (no indirect DMA):
  - "fast" region: each batch's first <=512 unmasked rows occupy exactly 4
    fixed 128-row chunks (padded with p=-sign(w)*15 rows whose softmax
    weight is exp(-sum|w|)~3e-4 and whose A rows are 0).
  - "leftover" region: rows beyond 512 per batch, packed into NLEFT shared
    chunks, batch-bound via host-built one-hot matrices (SPMD-uniform).

Transposed layout for the score phase: p is stored [dp, c, row] with
HID = c*128 + dp, so ACT computes tanh(p + att_h) in ONE pass using the
per-partition bias operand (att_hT column) -- no DVE add, no broadcast
matmul.  Scores are PE matmuls with the tanh tile as the (FWL fp8)
stationary operand and w columns as an N=1 moving operand, accumulated
over the 4 d-chunks in PSUM.  exp on ACT (batched 8 chunks).  Weighted
sums are PE matmuls (M=1 e-column for fast chunks, M=16 one-hot*e for
leftover chunks) accumulating [16, 1024] + sums in one PSUM region.

dtypes: p/A/tanh in fp8 e3m4 (numpy-simulated rel err 8.6e-3 < 2e-2),
e/one-hots in bf16, att_h path in f32r/f32.
"""

import os
import sys
from contextlib import ExitStack

import numpy as np

for _p in (
    "/root/.axon_site",
    "/root/.axon_site/_ro/trn_rl_repo",
    "/root/.axon_site/_ro/pypackages",
    "/opt/trn_rl_repo",
):
    if os.path.isdir(_p) and _p not in sys.path:
        sys.path.append(_p)

import ml_dtypes
import concourse.bass as bass
import concourse.tile as tile
from concourse import bacc, mybir
from concourse.bass_utils import run_bass_kernel_spmd

B, S, RNN, HID = 128, 1024, 1024, 512
NCORES = 8
BS = B // NCORES          # batches per core
P = 128
NB = 4                    # fast chunks per batch (512 rows)
FAST_ROWS = NB * P        # 512
F32 = mybir.dt.float32
F32R = mybir.dt.float32r
BF16 = mybir.dt.bfloat16
E3 = mybir.dt.float8e3
NP_E3 = ml_dtypes.float8_e3m4
NP_BF16 = ml_dtypes.bfloat16

_PROG_CACHE = {}


def _build_program(nleft: int):
    """nleft: number of leftover 128-row chunks (shared across cores)."""
    key = nleft
    if key in _PROG_CACHE:
        return _PROG_CACHE[key]
    nch = nleft + BS * NB          # total chunks
    rt = nch * P                   # total packed rows
    lrows = nleft * P
    ngroups = (nch + 7) // 8

    nc = bacc.Bacc("TRN2", target_bir_lowering=False, debug=False, num_devices=NCORES)

    pT_d = nc.dram_tensor("pT", [P, 4, rt], E3, kind="ExternalInput").ap()
    A_d = nc.dram_tensor("A", [nch, P, RNN], E3, kind="ExternalInput").ap()
    hT_d = nc.dram_tensor("hT", [RNN, BS], F32R, kind="ExternalInput").ap()
    wT_d = nc.dram_tensor("wT", [RNN, HID], F32R, kind="ExternalInput").ap()
    bias_d = nc.dram_tensor("bias", [1, HID], F32, kind="ExternalInput").ap()
    waT_d = nc.dram_tensor("waT", [P, 4], BF16, kind="ExternalInput").ap()
    ones_d = nc.dram_tensor("ones", [P, 1], BF16, kind="ExternalInput").ap()
    ident_d = nc.dram_tensor("ident", [BS, BS], BF16, kind="ExternalInput").ap()
    ohL_d = nc.dram_tensor("ohL", [BS, lrows], BF16, kind="ExternalInput").ap()
    ohLT_d = nc.dram_tensor("ohLT", [P, nleft, BS], BF16, kind="ExternalInput").ap()
    cm_d = nc.dram_tensor("cm", [P, BS, BS], BF16, kind="ExternalInput").ap()
    out_d = nc.dram_tensor("out", [BS, RNN], F32, kind="ExternalOutput").ap()

    with tile.TileContext(nc) as tc, ExitStack() as ctx:
        res_pool = ctx.enter_context(tc.tile_pool(name="res", bufs=1))
        small = ctx.enter_context(tc.tile_pool(name="small", bufs=4))
        ps_hold = ctx.enter_context(tc.tile_pool(name="ps_hold", bufs=1, space="PSUM"))
        ps_score = ctx.enter_context(tc.tile_pool(name="ps_score", bufs=2, space="PSUM"))
        ps_bc = ctx.enter_context(tc.tile_pool(name="ps_bc", bufs=2, space="PSUM"))
        ps_setup = ctx.enter_context(tc.tile_pool(name="ps_setup", bufs=1, space="PSUM"))

        # ---------- constant loads (first on the queue) ----------
        hT_sb = res_pool.tile([P, RNN // P, BS], F32R, tag="hT_sb")
        nc.sync.dma_start(out=hT_sb, in_=hT_d.rearrange("(j p) b -> p j b", p=P))
        wT_sb = res_pool.tile([P, RNN // P, HID], F32R, tag="wT_sb")
        nc.sync.dma_start(out=wT_sb, in_=wT_d.rearrange("(j p) d -> p j d", p=P))
        bias_sb = res_pool.tile([1, HID], F32, tag="bias_sb")
        nc.sync.dma_start(out=bias_sb, in_=bias_d)
        waT_sb = res_pool.tile([P, 4], BF16, tag="waT_sb")
        nc.sync.dma_start(out=waT_sb, in_=waT_d)
        ones_sb = res_pool.tile([P, 1], BF16, tag="ones_sb")
        nc.sync.dma_start(out=ones_sb, in_=ones_d)
        ident_sb = res_pool.tile([BS, BS], BF16, tag="ident_sb")
        nc.sync.dma_start(out=ident_sb, in_=ident_d)
        ohL_sb = res_pool.tile([BS, lrows], BF16, tag="ohL_sb")
        nc.sync.dma_start(out=ohL_sb, in_=ohL_d)
        ohLT_sb = res_pool.tile([P, nleft, BS], BF16, tag="ohLT_sb")
        nc.sync.dma_start(out=ohLT_sb, in_=ohLT_d)
        cm_sb = res_pool.tile([P, BS, BS], BF16, tag="cm_sb")
        nc.sync.dma_start(out=cm_sb, in_=cm_d)
        onesb_sb = res_pool.tile([1, BS], F32, tag="onesb_sb")
        nc.vector.memset(onesb_sb, 1.0)

        # ---------- bulk data loads: leftover first, then per batch ----------
        pTL = res_pool.tile([P, 4, lrows], E3, tag="pTL")
        nc.sync.dma_start(out=pTL, in_=pT_d[:, :, 0:lrows])
        AL = res_pool.tile([P, nleft, RNN], E3, tag="AL")
        nc.sync.dma_start(out=AL, in_=A_d[0:nleft].rearrange("c p d -> p c d"))
        pTF = []
        AF = []
        for b in range(BS):
            r0 = lrows + b * FAST_ROWS
            t = res_pool.tile([P, 4, FAST_ROWS], E3, tag=f"pTF{b}", name=f"pTF{b}")
            nc.sync.dma_start(out=t, in_=pT_d[:, :, r0 : r0 + FAST_ROWS])
            pTF.append(t)
            c0 = nleft + b * NB
            a = res_pool.tile([P, NB, RNN], E3, tag=f"AF{b}", name=f"AF{b}")
            nc.sync.dma_start(out=a, in_=A_d[c0 : c0 + NB].rearrange("c p d -> p c d"))
            AF.append(a)

        # ---------- setup: att_h = h @ W.T + bias ; att_hT ----------
        att_ps = ps_setup.tile([BS, HID], F32, tag="att_ps")
        for j in range(RNN // P):
            nc.tensor.matmul(
                out=att_ps,
                lhsT=hT_sb[:, j, :],
                rhs=wT_sb[:, j, :],
                start=(j == 0),
                stop=False,
            )
        nc.tensor.matmul(out=att_ps, lhsT=onesb_sb, rhs=bias_sb, start=False, stop=True)
        att_h_sb = res_pool.tile([BS, HID], BF16, tag="att_h_sb")
        nc.scalar.copy(att_h_sb, att_ps)

        # att_hT[dp, c, b] = att_h[b, c*128+dp]  via 4 PE transposes
        attT_ps = ps_setup.tile([P, 4, BS], BF16, tag="attT_ps")
        for c in range(4):
            nc.tensor.transpose(
                out=attT_ps[:, c, :],
                in_=att_h_sb[:, c * P : (c + 1) * P],
                identity=ident_sb,
            )
        att_hT_sb = res_pool.tile([P, 4, BS], BF16, tag="att_hT_sb")
        nc.scalar.copy(att_hT_sb, attT_ps)

        # tanh storage: leftover + one tile per batch
        tanhL = res_pool.tile([P, 4, lrows], E3, tag="tanhL")
        tanhF = [res_pool.tile([P, 4, FAST_ROWS], E3, tag=f"tF{b}", name=f"tF{b}") for b in range(BS)]
        e_sb = res_pool.tile([P, ngroups * 8], BF16, tag="e_sb")

        hold = ps_hold.tile([P, RNN], F32)
        res_ps = hold[0:BS, :]
        sums_ps = hold[64 : 64 + BS, 0:1]

        # ---------- helpers ----------
        def tanh_ap(k):
            """(lhsT source) tanh tile + row slice for chunk k, d-chunk c."""
            if k < nleft:
                return tanhL, k * P
            kf = k - nleft
            return tanhF[kf // NB], (kf % NB) * P

        def emit_leftover_pre(l):
            # bc = one-hot broadcast of att_h to leftover rows (transposed)
            bc = ps_bc.tile([P, 4, P], F32, tag="bc")
            for c in range(4):
                nc.tensor.matmul(
                    out=bc[:, c, :],
                    lhsT=att_h_sb[:, c * P : (c + 1) * P],
                    rhs=ohL_sb[:, l * P : (l + 1) * P],
                    start=True,
                    stop=True,
                )
            x = small.tile([P, 4, P], BF16, tag="xL")
            nc.vector.tensor_add(x, bc, pTL[:, :, l * P : (l + 1) * P])
            nc.scalar.activation(
                out=tanhL[:, :, l * P : (l + 1) * P],
                in_=x,
                func=mybir.ActivationFunctionType.Tanh,
            )

        def emit_fast_tanh(b):
            for c in range(4):
                nc.scalar.activation(
                    out=tanhF[b][:, c, :],
                    in_=pTF[b][:, c, :],
                    func=mybir.ActivationFunctionType.Tanh,
                    bias=att_hT_sb[:, c, b : b + 1],
                )

        def emit_score(k, sc_tile, slot):
            t, r0 = tanh_ap(k)
            for c in range(4):
                nc.tensor.matmul(
                    out=sc_tile[:, slot : slot + 1],
                    lhsT=t[:, c, r0 : r0 + P],
                    rhs=waT_sb[:, c : c + 1],
                    start=(c == 0),
                    stop=(c == 3),
                )

        def emit_res(k):
            st = k == 0
            sp = k == nch - 1
            if k < nleft:
                oh_src = ohLT_sb[:, k, :]
                rhs_t, j = AL, k
            else:
                kf = k - nleft
                b, j = kf // NB, kf % NB
                oh_src = cm_sb[:, b, :]
                rhs_t = AF[b]
            ohw = small.tile([P, BS], BF16, tag="ohw")
            nc.vector.tensor_mul(
                ohw, oh_src, e_sb[:, k : k + 1].broadcast_to([P, BS])
            )
            nc.tensor.matmul(
                out=res_ps[:, 0:512], lhsT=ohw, rhs=rhs_t[:, j, 0:512], start=st, stop=sp
            )
            nc.tensor.matmul(
                out=res_ps[:, 512:1024], lhsT=ohw, rhs=rhs_t[:, j, 512:1024],
                start=st, stop=sp,
            )
            nc.tensor.matmul(out=sums_ps, lhsT=ohw, rhs=ones_sb, start=st, stop=sp)

        # ---------- main pipeline ----------
        LAG = 2
        pend = []
        tanh_done = set()
        for g in range(ngroups):
            ks = [k for k in range(8 * g, min(8 * g + 8, nch))]
            sc = ps_score.tile([P, 8], F32, tag="score")
            for k in ks:
                if k < nleft:
                    emit_leftover_pre(k)
                else:
                    b = (k - nleft) // NB
                    if b not in tanh_done:
                        tanh_done.add(b)
                        emit_fast_tanh(b)
                emit_score(k, sc, k - 8 * g)
            nc.scalar.activation(
                out=e_sb[:, 8 * g : 8 * g + len(ks)],
                in_=sc[:, 0 : len(ks)],
                func=mybir.ActivationFunctionType.Exp,
            )
            pend.append(ks)
            if len(pend) > LAG:
                for k in pend.pop(0):
                    emit_res(k)
        for ks in pend:
            for k in ks:
                emit_res(k)

        # ---------- normalize + store ----------
        recip_sb = res_pool.tile([BS, 1], F32, tag="recip_sb")
        nc.vector.reciprocal(recip_sb, sums_ps)
        out_sb = res_pool.tile([BS, RNN], F32, tag="out_sb")
        nc.vector.tensor_scalar_mul(out=out_sb, in0=res_ps, scalar1=recip_sb)
        nc.sync.dma_start(out=out_d, in_=out_sb)

    nc.compile()
    _PROG_CACHE[key] = nc
    return nc


def _pack_core(m, p_flat, A_flat, mask, wa):
    """Host-side packing for core m. Returns (rows_fast[BS,512], left_rows, left_b)."""
    mask_m = mask[m * BS : (m + 1) * BS]
    fast = np.full((BS, FAST_ROWS), -1, np.int64)
    left_rows = []
    left_b = []
    for b in range(BS):
        idx = np.flatnonzero(~mask_m[b])
        n = min(len(idx), FAST_ROWS)
        fast[b, :n] = b * S + idx[:n]
        if len(idx) > FAST_ROWS:
            extra = b * S + idx[FAST_ROWS:]
            left_rows.append(extra)
            left_b.append(np.full(len(extra), b, np.int64))
    left_rows = np.concatenate(left_rows) if left_rows else np.empty(0, np.int64)
    left_b = np.concatenate(left_b) if left_b else np.empty(0, np.int64)
    return fast, left_rows, left_b


def run(inputs, trace: bool = False, trace_kwargs: dict | None = None, **_ignored):
    h = np.asarray(inputs["h"], dtype=np.float32)
    A = np.asarray(inputs["att_feats"], dtype=np.float32)
    p = np.asarray(inputs["p_att_feats"], dtype=np.float32)
    mask = np.asarray(inputs["mask"]).astype(bool)
    W = np.asarray(inputs["W_h2att"], dtype=np.float32)
    bh = np.asarray(inputs["b_h2att"], dtype=np.float32)
    wa = np.asarray(inputs["w_alpha"], dtype=np.float32)

    packs = [
        _pack_core(m, None, None, mask, wa) for m in range(NCORES)
    ]
    nleft = max(1, max((len(lr) + P - 1) // P for (_, lr, _) in packs))
    nch = nleft + BS * NB
    rt = nch * P
    lrows = nleft * P

    pad_row = (-np.sign(wa) * 15.0).astype(np.float32)
    pad_row[wa == 0] = -15.0

    shared = {
        "wT": np.ascontiguousarray(W.T),
        "bias": np.ascontiguousarray(bh[None, :]),
        "waT": np.ascontiguousarray(wa.reshape(4, P).T).astype(NP_BF16),
        "ones": np.ones((P, 1), NP_BF16),
        "ident": np.eye(BS, dtype=np.float32).astype(NP_BF16),
        "cm": np.ascontiguousarray(
            np.broadcast_to(np.eye(BS, dtype=np.float32), (P, BS, BS)).transpose(0, 2, 1)
        ).astype(NP_BF16),
    }

    in_maps = []
    for m in range(NCORES):
        fast, left_rows, left_b = packs[m]
        sl = slice(m * BS, (m + 1) * BS)
        p_m = p[sl].reshape(BS * S, HID)
        A_m = A[sl].reshape(BS * S, RNN)

        rows = np.full(rt, -1, np.int64)
        rows[:len(left_rows)] = left_rows
        rows[lrows:] = fast.reshape(-1)
        valid = rows >= 0

        pg = np.empty((rt, HID), np.float32)
        pg[valid] = p_m[rows[valid]]
        pg[~valid] = pad_row
        pT_host = np.ascontiguousarray(
            pg.reshape(rt, 4, P).transpose(2, 1, 0)
        ).astype(NP_E3)

        Ag = np.zeros((rt, RNN), np.float32)
        Ag[valid] = A_m[rows[valid]]
        A_host = np.ascontiguousarray(Ag.reshape(nch, P, RNN)).astype(NP_E3)

        ohL = np.zeros((BS, lrows), np.float32)
        ohL[left_b, np.arange(len(left_b))] = 1.0
        ohLT = np.ascontiguousarray(
            ohL.T.reshape(nleft, P, BS).transpose(1, 0, 2)
        ).astype(NP_BF16)

        in_map = dict(shared)
        in_map.update(
            {
                "pT": pT_host,
                "A": A_host,
                "hT": np.ascontiguousarray(h[sl].T),
                "ohL": ohL.astype(NP_BF16),
                "ohLT": ohLT,
            }
        )
        in_maps.append(in_map)

    nc = _build_program(nleft)
    br = run_bass_kernel_spmd(
        nc,
        in_maps,
        core_ids=list(range(NCORES)),
        trace=trace,
        **(trace_kwargs or {}),
    )
    out = np.concatenate([br.results[m]["out"] for m in range(NCORES)], axis=0)
    out = out.astype(np.float32)

    # Fully-masked batches: reference softmax degenerates to uniform weights.
    dead = np.flatnonzero(mask.all(axis=1))
    for b in dead:
        out[b] = A[b].mean(axis=0, dtype=np.float64).astype(np.float32)
    return out, br


def kernel(**inputs) -> np.ndarray:
    out, _ = run(inputs, trace=False)
    return out


# revision 8
# speedup vs baseline: 1.7282x; 1.0858x over previous
"""Trainium2 Bass kernel for nn_Attention2 (sparse additive attention), v2.

Math (per batch b):
    att_h  = h @ W_h2att.T + b_h2att                       [HID]
    x      = p_att_feats[b] + att_h                        [S, HID]
    scores = tanh(x) @ w_alpha   (+ b_alpha, cancels)      [S]
    scores = where(mask, -1e8, scores)
    w      = softmax(scores)
    out[b] = w @ att_feats[b]                              [RNN]

Data-parallel over batch (16 batches/core on 8 cores). Only unmasked rows
are processed; the host packs them densely (no indirect DMA):
  - "fast" region: each batch's first <=512 unmasked rows occupy exactly 4
    fixed 128-row chunks (padded with p=-sign(w)*15 rows whose softmax
    weight is exp(-sum|w|)~3e-4 and whose A rows are 0).
  - "leftover" region: rows beyond 512 per batch, packed into NLEFT shared
    chunks, batch-bound via host-built one-hot matrices (SPMD-uniform).

Transposed layout for the score phase: p is stored [dp, c, row] with
HID = c*128 + dp, so ACT computes tanh(p + att_h) in ONE pass using the
per-partition bias operand (att_hT column) -- no DVE add, no broadcast
matmul.  Scores are PE matmuls with the tanh tile as the (FWL fp8)
stationary operand and w columns as an N=1 moving operand, accumulated
over the 4 d-chunks in PSUM.  exp on ACT (batched 8 chunks).  Weighted
sums are PE matmuls (M=1 e-column for fast chunks, M=16 one-hot*e for
leftover chunks) accumulating [16, 1024] + sums in one PSUM region.

dtypes: p/A/tanh in fp8 e3m4 (numpy-simulated rel err 8.6e-3 < 2e-2),
e/one-hots in bf16, att_h path in f32r/f32.
"""

import os
import sys
from contextlib import ExitStack

import numpy as np

for _p in (
    "/root/.axon_site",
    "/root/.axon_site/_ro/trn_rl_repo",
    "/root/.axon_site/_ro/pypackages",
    "/opt/trn_rl_repo",
):
    if os.path.isdir(_p) and _p not in sys.path:
        sys.path.append(_p)

import ml_dtypes
import concourse.bass as bass
import concourse.tile as tile
from concourse import bacc, mybir
from concourse.bass_utils import run_bass_kernel_spmd

B, S, RNN, HID = 128, 1024, 1024, 512
NCORES = 8
BS = B // NCORES          # batches per core
P = 128
NB = 4                    # fast chunks per batch (512 rows)
FAST_ROWS = NB * P        # 512
F32 = mybir.dt.float32
F32R = mybir.dt.float32r
BF16 = mybir.dt.bfloat16
E3 = mybir.dt.float8e3
NP_E3 = ml_dtypes.float8_e3m4
NP_BF16 = ml_dtypes.bfloat16

_PROG_CACHE = {}


def _build_program(nleft: int):
    """nleft: number of leftover 128-row chunks (shared across cores)."""
    key = nleft
    if key in _PROG_CACHE:
        return _PROG_CACHE[key]
    nch = nleft + BS * NB          # total chunks
    rt = nch * P                   # total packed rows
    lrows = nleft * P
    ngroups = (nch + 7) // 8

    nc = bacc.Bacc("TRN2", target_bir_lowering=False, debug=False, num_devices=NCORES)

    pT_d = nc.dram_tensor("pT", [P, 4, rt], E3, kind="ExternalInput").ap()
    A_d = nc.dram_tensor("A", [nch, P, RNN], E3, kind="ExternalInput").ap()
    hT_d = nc.dram_tensor("hT", [RNN, BS], BF16, kind="ExternalInput").ap()
    wT_d = nc.dram_tensor("wT", [RNN, HID], BF16, kind="ExternalInput").ap()
    bias_d = nc.dram_tensor("bias", [1, HID], F32, kind="ExternalInput").ap()
    waT_d = nc.dram_tensor("waT", [P, 4], BF16, kind="ExternalInput").ap()
    ones_d = nc.dram_tensor("ones", [P, 1], BF16, kind="ExternalInput").ap()
    ident_d = nc.dram_tensor("ident", [BS, BS], BF16, kind="ExternalInput").ap()
    ohL_d = nc.dram_tensor("ohL", [BS, lrows], BF16, kind="ExternalInput").ap()
    ohLT_d = nc.dram_tensor("ohLT", [P, nleft, BS], BF16, kind="ExternalInput").ap()
    cm_d = nc.dram_tensor("cm", [P, BS, BS], BF16, kind="ExternalInput").ap()
    out_d = nc.dram_tensor("out", [BS, RNN], F32, kind="ExternalOutput").ap()

    with tile.TileContext(nc) as tc, ExitStack() as ctx:
        res_pool = ctx.enter_context(tc.tile_pool(name="res", bufs=1))
        small = ctx.enter_context(tc.tile_pool(name="small", bufs=4))
        ps_hold = ctx.enter_context(tc.tile_pool(name="ps_hold", bufs=1, space="PSUM"))
        ps_score = ctx.enter_context(tc.tile_pool(name="ps_score", bufs=2, space="PSUM"))
        ps_bc = ctx.enter_context(tc.tile_pool(name="ps_bc", bufs=2, space="PSUM"))
        ps_setup = ctx.enter_context(tc.tile_pool(name="ps_setup", bufs=1, space="PSUM"))

        # ---------- constant loads (first on the queue) ----------
        hT_sb = res_pool.tile([P, RNN // P, BS], BF16, tag="hT_sb")
        nc.sync.dma_start(out=hT_sb, in_=hT_d.rearrange("(j p) b -> p j b", p=P))
        wT_sb = res_pool.tile([P, RNN // P, HID], BF16, tag="wT_sb")
        nc.sync.dma_start(out=wT_sb, in_=wT_d.rearrange("(j p) d -> p j d", p=P))
        bias_sb = res_pool.tile([1, HID], F32, tag="bias_sb")
        nc.sync.dma_start(out=bias_sb, in_=bias_d)
        waT_sb = res_pool.tile([P, 4], BF16, tag="waT_sb")
        nc.sync.dma_start(out=waT_sb, in_=waT_d)
        ones_sb = res_pool.tile([P, 1], BF16, tag="ones_sb")
        nc.sync.dma_start(out=ones_sb, in_=ones_d)
        ident_sb = res_pool.tile([BS, BS], BF16, tag="ident_sb")
        nc.sync.dma_start(out=ident_sb, in_=ident_d)
        ohL_sb = res_pool.tile([BS, lrows], BF16, tag="ohL_sb")
        nc.sync.dma_start(out=ohL_sb, in_=ohL_d)
        ohLT_sb = res_pool.tile([P, nleft, BS], BF16, tag="ohLT_sb")
        nc.sync.dma_start(out=ohLT_sb, in_=ohLT_d)
        cm_sb = res_pool.tile([P, BS, BS], BF16, tag="cm_sb")
        nc.sync.dma_start(out=cm_sb, in_=cm_d)
        onesb_sb = res_pool.tile([1, BS], F32, tag="onesb_sb")
        nc.vector.memset(onesb_sb, 1.0)

        # ---------- bulk data loads: leftover first, then per batch ----------
        pTL = res_pool.tile([P, 4, lrows], E3, tag="pTL")
        nc.gpsimd.dma_start(out=pTL, in_=pT_d[:, :, 0:lrows])
        AL = res_pool.tile([P, nleft, RNN], E3, tag="AL")
        nc.gpsimd.dma_start(out=AL, in_=A_d[0:nleft].rearrange("c p d -> p c d"))
        pTF = []
        AF = []
        for b in range(BS):
            r0 = lrows + b * FAST_ROWS
            t = res_pool.tile([P, 4, FAST_ROWS], E3, tag=f"pTF{b}", name=f"pTF{b}")
            nc.gpsimd.dma_start(out=t, in_=pT_d[:, :, r0 : r0 + FAST_ROWS])
            pTF.append(t)
            c0 = nleft + b * NB
            a = res_pool.tile([P, NB, RNN], E3, tag=f"AF{b}", name=f"AF{b}")
            nc.gpsimd.dma_start(out=a, in_=A_d[c0 : c0 + NB].rearrange("c p d -> p c d"))
            AF.append(a)

        # ---------- setup: att_h = h @ W.T + bias ; att_hT ----------
        att_ps = ps_setup.tile([BS, HID], F32, tag="att_ps")
        for j in range(RNN // P):
            nc.tensor.matmul(
                out=att_ps,
                lhsT=hT_sb[:, j, :],
                rhs=wT_sb[:, j, :],
                start=(j == 0),
                stop=False,
            )
        nc.tensor.matmul(out=att_ps, lhsT=onesb_sb, rhs=bias_sb, start=False, stop=True)
        att_h_sb = res_pool.tile([BS, HID], BF16, tag="att_h_sb")
        nc.scalar.copy(att_h_sb, att_ps)

        # att_hT[dp, c, b] = att_h[b, c*128+dp]  via 4 PE transposes
        attT_ps = ps_setup.tile([P, 4, BS], BF16, tag="attT_ps")
        for c in range(4):
            nc.tensor.transpose(
                out=attT_ps[:, c, :],
                in_=att_h_sb[:, c * P : (c + 1) * P],
                identity=ident_sb,
            )
        att_hT_sb = res_pool.tile([P, 4, BS], BF16, tag="att_hT_sb")
        nc.scalar.copy(att_hT_sb, attT_ps)

        # tanh storage: leftover + one tile per batch
        tanhL = res_pool.tile([P, 4, lrows], E3, tag="tanhL")
        tanhF = [res_pool.tile([P, 4, FAST_ROWS], E3, tag=f"tF{b}", name=f"tF{b}") for b in range(BS)]
        e_sb = res_pool.tile([P, ngroups * 8], BF16, tag="e_sb")

        hold = ps_hold.tile([P, RNN], F32)
        res_ps = hold[0:BS, :]
        sums_ps = hold[64 : 64 + BS, 0:1]

        # ---------- helpers ----------
        def tanh_ap(k):
            """(lhsT source) tanh tile + row slice for chunk k, d-chunk c."""
            if k < nleft:
                return tanhL, k * P
            kf = k - nleft
            return tanhF[kf // NB], (kf % NB) * P

        def emit_leftover_pre(l):
            # bc = one-hot broadcast of att_h to leftover rows (transposed)
            bc = ps_bc.tile([P, 4, P], F32, tag="bc")
            for c in range(4):
                nc.tensor.matmul(
                    out=bc[:, c, :],
                    lhsT=att_h_sb[:, c * P : (c + 1) * P],
                    rhs=ohL_sb[:, l * P : (l + 1) * P],
                    start=True,
                    stop=True,
                )
            x = small.tile([P, 4, P], BF16, tag="xL")
            nc.vector.tensor_add(x, bc, pTL[:, :, l * P : (l + 1) * P])
            nc.scalar.activation(
                out=tanhL[:, :, l * P : (l + 1) * P],
                in_=x,
                func=mybir.ActivationFunctionType.Tanh,
            )

        def emit_fast_tanh(b):
            for c in range(4):
                nc.scalar.activation(
                    out=tanhF[b][:, c, :],
                    in_=pTF[b][:, c, :],
                    func=mybir.ActivationFunctionType.Tanh,
                    bias=att_hT_sb[:, c, b : b + 1],
                )

        def emit_score(k, sc_tile, slot):
            t, r0 = tanh_ap(k)
            for c in range(4):
                nc.tensor.matmul(
                    out=sc_tile[:, slot : slot + 1],
                    lhsT=t[:, c, r0 : r0 + P],
                    rhs=waT_sb[:, c : c + 1],
                    start=(c == 0),
                    stop=(c == 3),
                )

        def emit_res(k):
            st = k == 0
            sp = k == nch - 1
            if k < nleft:
                oh_src = ohLT_sb[:, k, :]
                rhs_t, j = AL, k
            else:
                kf = k - nleft
                b, j = kf // NB, kf % NB
                oh_src = cm_sb[:, b, :]
                rhs_t = AF[b]
            ohw = small.tile([P, BS], BF16, tag="ohw")
            nc.vector.tensor_mul(
                ohw, oh_src, e_sb[:, k : k + 1].broadcast_to([P, BS])
            )
            nc.tensor.matmul(
                out=res_ps[:, 0:512], lhsT=ohw, rhs=rhs_t[:, j, 0:512], start=st, stop=sp
            )
            nc.tensor.matmul(
                out=res_ps[:, 512:1024], lhsT=ohw, rhs=rhs_t[:, j, 512:1024],
                start=st, stop=sp,
            )
            nc.tensor.matmul(out=sums_ps, lhsT=ohw, rhs=ones_sb, start=st, stop=sp)

        # ---------- main pipeline ----------
        LAG = 0
        pend = []
        tanh_done = set()
        for g in range(ngroups):
            ks = [k for k in range(8 * g, min(8 * g + 8, nch))]
            sc = ps_score.tile([P, 8], F32, tag="score")
            for k in ks:
                if k < nleft:
                    emit_leftover_pre(k)
                else:
                    b = (k - nleft) // NB
                    if b not in tanh_done:
                        tanh_done.add(b)
                        emit_fast_tanh(b)
                emit_score(k, sc, k - 8 * g)
            nc.scalar.activation(
                out=e_sb[:, 8 * g : 8 * g + len(ks)],
                in_=sc[:, 0 : len(ks)],
                func=mybir.ActivationFunctionType.Exp,
            )
            pend.append(ks)
            if len(pend) > LAG:
                for k in pend.pop(0):
                    emit_res(k)
        for ks in pend:
            for k in ks:
                emit_res(k)

        # ---------- normalize + store ----------
        recip_sb = res_pool.tile([BS, 1], F32, tag="recip_sb")
        nc.vector.reciprocal(recip_sb, sums_ps)
        out_sb = res_pool.tile([BS, RNN], F32, tag="out_sb")
        nc.vector.tensor_scalar_mul(out=out_sb, in0=res_ps, scalar1=recip_sb)
        nc.sync.dma_start(out=out_d, in_=out_sb)

    nc.compile()
    _PROG_CACHE[key] = nc
    return nc


def _pack_core(m, p_flat, A_flat, mask, wa):
    """Host-side packing for core m. Returns (rows_fast[BS,512], left_rows, left_b)."""
    mask_m = mask[m * BS : (m + 1) * BS]
    fast = np.full((BS, FAST_ROWS), -1, np.int64)
    left_rows = []
    left_b = []
    for b in range(BS):
        idx = np.flatnonzero(~mask_m[b])
        n = min(len(idx), FAST_ROWS)
        fast[b, :n] = b * S + idx[:n]
        if len(idx) > FAST_ROWS:
            extra = b * S + idx[FAST_ROWS:]
            left_rows.append(extra)
            left_b.append(np.full(len(extra), b, np.int64))
    left_rows = np.concatenate(left_rows) if left_rows else np.empty(0, np.int64)
    left_b = np.concatenate(left_b) if left_b else np.empty(0, np.int64)
    return fast, left_rows, left_b


def run(inputs, trace: bool = False, trace_kwargs: dict | None = None, **_ignored):
    h = np.asarray(inputs["h"], dtype=np.float32)
    A = np.asarray(inputs["att_feats"], dtype=np.float32)
    p = np.asarray(inputs["p_att_feats"], dtype=np.float32)
    mask = np.asarray(inputs["mask"]).astype(bool)
    W = np.asarray(inputs["W_h2att"], dtype=np.float32)
    bh = np.asarray(inputs["b_h2att"], dtype=np.float32)
    wa = np.asarray(inputs["w_alpha"], dtype=np.float32)

    packs = [
        _pack_core(m, None, None, mask, wa) for m in range(NCORES)
    ]
    nleft = max(1, max((len(lr) + P - 1) // P for (_, lr, _) in packs))
    nch = nleft + BS * NB
    rt = nch * P
    lrows = nleft * P

    pad_row = (-np.sign(wa) * 15.0).astype(np.float32)
    pad_row[wa == 0] = -15.0

    shared = {
        "wT": np.ascontiguousarray(W.T).astype(NP_BF16),
        "bias": np.ascontiguousarray(bh[None, :]),
        "waT": np.ascontiguousarray(wa.reshape(4, P).T).astype(NP_BF16),
        "ones": np.ones((P, 1), NP_BF16),
        "ident": np.eye(BS, dtype=np.float32).astype(NP_BF16),
        "cm": np.ascontiguousarray(
            np.broadcast_to(np.eye(BS, dtype=np.float32), (P, BS, BS)).transpose(0, 2, 1)
        ).astype(NP_BF16),
    }

    in_maps = []
    for m in range(NCORES):
        fast, left_rows, left_b = packs[m]
        sl = slice(m * BS, (m + 1) * BS)
        p_m = p[sl].reshape(BS * S, HID)
        A_m = A[sl].reshape(BS * S, RNN)

        rows = np.full(rt, -1, np.int64)
        rows[:len(left_rows)] = left_rows
        rows[lrows:] = fast.reshape(-1)
        valid = rows >= 0

        pg = np.empty((rt, HID), np.float32)
        pg[valid] = p_m[rows[valid]]
        pg[~valid] = pad_row
        pT_host = np.ascontiguousarray(
            pg.reshape(rt, 4, P).transpose(2, 1, 0)
        ).astype(NP_E3)

        Ag = np.zeros((rt, RNN), np.float32)
        Ag[valid] = A_m[rows[valid]]
        A_host = np.ascontiguousarray(Ag.reshape(nch, P, RNN)).astype(NP_E3)

        ohL = np.zeros((BS, lrows), np.float32)
        ohL[left_b, np.arange(len(left_b))] = 1.0
        ohLT = np.ascontiguousarray(
            ohL.T.reshape(nleft, P, BS).transpose(1, 0, 2)
        ).astype(NP_BF16)

        in_map = dict(shared)
        in_map.update(
            {
                "pT": pT_host,
                "A": A_host,
                "hT": np.ascontiguousarray(h[sl].T).astype(NP_BF16),
                "ohL": ohL.astype(NP_BF16),
                "ohLT": ohLT,
            }
        )
        in_maps.append(in_map)

    nc = _build_program(nleft)
    br = run_bass_kernel_spmd(
        nc,
        in_maps,
        core_ids=list(range(NCORES)),
        trace=trace,
        **(trace_kwargs or {}),
    )
    out = np.concatenate([br.results[m]["out"] for m in range(NCORES)], axis=0)
    out = out.astype(np.float32)

    # Fully-masked batches: reference softmax degenerates to uniform weights.
    dead = np.flatnonzero(mask.all(axis=1))
    for b in dead:
        out[b] = A[b].mean(axis=0, dtype=np.float64).astype(np.float32)
    return out, br


def kernel(**inputs) -> np.ndarray:
    out, _ = run(inputs, trace=False)
    return out
